# revision 26
# baseline (speedup 1.0000x reference)
"""Trainium2 Bass kernel for the blended-MoE actor network.

Math: reference computes, per sample,
    g1 = relu(bw1 @ s + bb1); g2 = relu(bw2 @ g1 + bb2)
    c  = softmax(bwo @ g2 + bbo)            # 2 experts
    h1 = relu(blend(W1_e, s)); h2 = relu(blend(W2_e, h1))
    mu = tanh(blend(Wm_e, h2))
with blend(W_e, x) = sum_e c_e (W_e x + b_e).

Since NE=2 and c0+c1=1:  c0 = sigmoid((bwo[0]-bwo[1]) @ g2 + dbo)  and
    blend(W_e, x) = W_1 x + b_1 + c0 * (dW x + db),  dW = W_0-W_1.
For L1/L2 the c0 * (dW x) term is computed by scaling the matmul INPUT
per-sample (x_c = C0 .* x) so both expert contributions accumulate into
one PSUM group. Rank-1 bias terms ride along: an appended ones-row on
the states makes row 376 of the scaled states equal c0 (feeding the db
column of the augmented diff weights); for L2 the bias diff is folded
into h1 via v solving dW2 v = db2 (h1c = C0 .* (h1 + v), base-path
constant compensated in the h2 bias).
The OUT layer blends at the output instead: one [128,34] stationary
tile [Wm1.T | dWm.T] yields both expert heads from a single matmul;
mu = tanh(y1 + c0*(yd + dbm) + bm1) with the inner term one
scalar_tensor_tensor op (per-partition dbm, tensor c0).

Layout: activations are [features, batch] on-chip (host pre-transposes
states and appends a ones row); batch tiles of N=512 (one PSUM bank per
matmul). The router logit-diff matmul uses wd replicated across 128
output columns so its PSUM output holds the logit diff in every
partition row: one sigmoid yields the broadcast C0 tile for free.

The per-tile dataflow is a serial chain (blend MLP -> C0 -> scaled
inputs -> expert layers), so instructions are emitted in an explicit
software-pipelined order across batch tiles -- the TensorEngine stream
interleaves expert layers of tiles t, t-1, t-2 with the blend MLP of
tile t+2, keeping the PE free of cross-engine round-trip stalls.

Engine balance per tile (measured ns): PE 22 matmuls ~4700, DVE
(h1 relu+v x2, h1c x2, sc chunk0, L3 blend x2) ~4000, GpSimd (sc
chunks 1/2) ~2600, ACT (g1/g2/h2 relu, sigmoid, tanh) ~3450.

Output is written bf16 in a tile-contiguous DRAM layout ([T*17, 512])
so the final DMA is a small contiguous burst instead of a slow
17-row strided scatter; the host reassembles.

Sharding: pure data parallel over 8 cores (batch 65536 -> 8 x 8192).
"""

import ml_dtypes
import numpy as np

import concourse.bass as bass
import concourse.mybir as mybir
import concourse.tile as tile
from concourse import bacc
from concourse.bass_utils import run_bass_kernel_spmd

N_CORES = 8
B = 65536
BS = B // N_CORES  # 8192 per core
NI = 376  # state features
NIA = NI + 1  # + ones row
NA = 17  # actions
BH = 128  # blending hidden
NT = 512  # batch tile (matmul free dim, one PSUM bank)
T = BS // NT  # 16 tiles per core

F32 = mybir.dt.float32
# bf16 compute: matmul streams 1 col/cycle warm (f32r measures 2), DVE
# tensor_tensor gets 2x mode, DMA bytes halve. rel err ~8e-3 << 2e-2.
DT = mybir.dt.bfloat16
DT_NP = ml_dtypes.bfloat16
F8 = mybir.dt.float8e4
F8_NP = ml_dtypes.float8_e4m3fn

AF = mybir.ActivationFunctionType
ALU = mybir.AluOpType
KCH = ((0, 128), (128, 256), (256, NIA))  # K chunks of the state dim


# ---------------------------------------------------------------- weights
# All stationary operands are packed into one [128, WCOLS] host array;
# each lhsT is a column slice [0:K, off:off+M]. Rows >= K are zero.


class _Pack:
    def __init__(self):
        self.cols = []
        self.off = 0

    def add(self, arr):  # arr [K, M] -> returns (off, K, M)
        k, m = arr.shape
        assert k <= 128
        a = np.zeros((128, m), np.float32)
        a[:k] = arr
        off = self.off
        self.cols.append(a)
        self.off += m
        return (off, k, m)

    def data(self):
        return np.concatenate(self.cols, axis=1)


def _prep_weights(p, bw1, bb1, bw2, bb2, bwo, bbo, ew1, eb1, ew2, eb2, ewm, ebm):
    d = {}
    # Pack is CHUNK-MAJOR: section k holds every stationary consuming state
    # chunk k (blend L1 + expert L1 base/diff), so the first matmuls are
    # gated on one small wk section + one state chunk instead of the whole
    # pack. Section 3 holds everything downstream of the state chunks.
    w1a = np.concatenate([bw1.T, bb1[None, :]], axis=0)  # [377, 128]
    e1b = np.concatenate([ew1[1].T, eb1[1][None, :]], axis=0)  # [377, 256]
    e1d = np.concatenate([(ew1[0] - ew1[1]).T, (eb1[0] - eb1[1])[None, :]], axis=0)
    d["bl1"] = [None] * 3
    d["e1b"] = [[None] * 3 for _ in range(2)]
    d["e1d"] = [[None] * 3 for _ in range(2)]
    d["splits"] = []
    for ci, (k0, k1) in enumerate(KCH):
        d["bl1"][ci] = p.add(w1a[k0:k1])
        for m in range(2):
            d["e1b"][m][ci] = p.add(e1b[k0:k1, m * 128 : m * 128 + 128])
            d["e1d"][m][ci] = p.add(e1d[k0:k1, m * 128 : m * 128 + 128])
        d["splits"].append(p.off)
    # --- section 3: blend L2/router + expert L2 / out weights
    d["bl2"] = [p.add(bw2.T)]
    d["bb2"] = p.add(bb2[:, None])
    # router logit diff, replicated to 128 output columns
    wd = (bwo[0] - bwo[1])[:, None]
    d["wd"] = [p.add(np.repeat(wd, 128, axis=1))]
    d["bd"] = p.add(np.full((128, 1), bbo[0] - bbo[1], np.float32))
    e2b = ew2[1].T  # [256, 128]
    e2d = (ew2[0] - ew2[1]).T
    d["e2b"] = [p.add(e2b[0:128]), p.add(e2b[128:256])]
    d["e2d"] = [p.add(e2d[0:128]), p.add(e2d[128:256])]
    # rank-1 bias c0*db2 folded into h1: with v solving (ew2[0]-ew2[1]) v =
    # db2 (min-norm), h1c = C0 .* (h1 + v) makes the e2d matmul reproduce
    # c0*db2 exactly; the constant (ew2[1] @ v) on the base path moves into
    # the h2-relu bias. Kills one K=1 matmul per tile.
    E = (ew2[0] - ew2[1]).astype(np.float64)  # [128, 256]
    v = np.linalg.lstsq(E, (eb2[0] - eb2[1]).astype(np.float64), rcond=None)[0]
    v = v.astype(np.float32)
    d["b2"] = p.add((eb2[1] - ew2[1] @ v)[:, None])
    # same rank-1 trick for the out layer (used by the tail tiles where the
    # output blend runs on the PE): vm solves dWm vm = dbm, h2c=(h2+vm)*c0
    Em = (ewm[0] - ewm[1]).astype(np.float64)  # [17, 128]
    vm = np.linalg.lstsq(Em, (ebm[0] - ebm[1]).astype(np.float64), rcond=None)[0]
    d["v2h"] = np.stack([v[0:128], v[128:256], vm.astype(np.float32)], axis=1)
    # expert out: both heads from one stationary tile [128, 49]; the diff
    # head sits at output partition 32 (PSUM reads must start 32-aligned)
    d["em"] = [p.add(np.concatenate(
        [ewm[1].T, np.zeros((128, 32 - NA), np.float32),
         (ewm[0] - ewm[1]).T], axis=1))]
    off_em = d["em"][0][0]
    d["em1"] = (off_em, 128, NA)
    d["emd"] = (off_em + 32, 128, NA)
    d["dbm"] = p.add((ebm[0] - ebm[1])[:, None])  # [17, 1]
    d["bm"] = p.add(ebm[1][:, None])  # [17, 1]
    return d


# ---------------------------------------------------------------- kernel


def _build(wd, wcols):
    nc = bacc.Bacc("TRN2", target_bir_lowering=False, debug=False,
                   num_devices=N_CORES)
    xs = nc.declare_dram_parameter("xs", [NIA, BS], DT, isOutput=False)
    wk = nc.declare_dram_parameter("wk", [128, wcols], DT, isOutput=False)
    vb = nc.declare_dram_parameter("vb", [128, 3], F32, isOutput=False)
    out = nc.declare_dram_parameter("out", [T * NA, NT], DT, isOutput=True)

    splits = wd["splits"]  # chunk-section boundaries in the pack

    with tile.TileContext(nc) as tc:
        with (
            tc.tile_pool(name="wpool", bufs=1) as wpool,
            tc.tile_pool(name="spool", bufs=3) as spool,
            tc.tile_pool(name="scpool", bufs=4) as scpool,
            tc.tile_pool(name="gpool", bufs=3) as gpool,
            tc.tile_pool(name="cpool", bufs=5) as cpool,
            tc.tile_pool(name="hpool", bufs=3) as hpool,
            tc.tile_pool(name="opool", bufs=3) as opool,
            tc.tile_pool(name="psum", bufs=1, space="PSUM") as pp,
        ):
            # Weight pack arrives as four tiles, chunk-major: section k holds
            # every stationary consuming state chunk k, so the first matmul
            # block is gated on ~160KB instead of the whole pack. All weight
            # DMAs ride the (otherwise idle) GpSimd queue so the Sync queue
            # serves the state-chunk DMAs back to back.
            scr = wpool.tile([128, NT], DT, tag="scr")
            nc.vector.memset(scr[:], 0.0)
            ends = splits + [wcols]
            begs = [0] + splits
            wkts = []
            for i, (b, e) in enumerate(zip(begs, ends)):
                wkts.append(wpool.tile([128, e - b], DT, name=f"wkt{i}",
                                       tag=f"wkt{i}"))
            nc.gpsimd.dma_start(wkts[0][:], wk[:, begs[0] : ends[0]])
            vbt = wpool.tile([128, 3], F32)

            def wk_dma(i, q=None):
                (q or nc.gpsimd).dma_start(wkts[i][:], wk[:, begs[i] : ends[i]])

            def W(desc):
                off, k, m = desc
                for i, e in enumerate(ends):
                    if off + m <= e:
                        return wkts[i][0:k, off - begs[i] : off - begs[i] + m]
                raise AssertionError("bad pack slice")

            # per-tile (t) and per-pair (p = t//2) live tensors; pairs are
            # 1024 wide so DMA and GpSimd muls run at half the instruction
            # and semaphore count. Pair 0 is loaded per-TILE instead: the
            # 16 DMA engines round-robin over every outstanding transfer,
            # so the first tile's data lands ~2x sooner when its transfers
            # are half-size and first in the queue.
            s = {}
            s0t = {}  # s0t[t][ci] for tiles 0/1
            sc = {}
            c0 = {}
            g1 = {}
            g2 = {}
            h1 = {}
            h1c = {}
            h2 = {}

            def psl(t):  # slice of tile t within its pair tile
                return slice((t % 2) * NT, (t % 2 + 1) * NT)

            def sv(t, ci):  # state chunk view for tile t
                if t < 2:
                    return s0t[t][ci][:]
                return s[t // 2][ci][:, psl(t)]

            def dma_q(ci):
                return nc.sync

            def dma_in_tile(t, chunks=(0, 1, 2), qs=None):
                s0t.setdefault(t, [None, None, None])
                for ci in chunks:
                    k0, k1 = KCH[ci]
                    st = spool.tile([k1 - k0, NT], DT, tag=f"st{t}_{ci}",
                                    name=f"st{t}_{ci}", bufs=1)
                    q = qs[ci] if qs else dma_q(ci)
                    q.dma_start(st[:], xs[k0:k1, t * NT : (t + 1) * NT])
                    s0t[t][ci] = st

            def dma_in(p, chunks=(0, 1, 2)):
                # pair p: columns [p*2NT, (p+1)*2NT); chunks may be staggered
                # across iterations to spread SBUF write-port contention
                if p not in s:
                    s[p] = [None, None, None]
                for ci in chunks:
                    k0, k1 = KCH[ci]
                    st = spool.tile([k1 - k0, 2 * NT], DT, tag=f"s{ci}",
                                    name=f"s{ci}_{p}", bufs=5)
                    dma_q(ci).dma_start(
                        st[:], xs[k0:k1, p * 2 * NT : (p + 1) * 2 * NT])
                    s[p][ci] = st

            # PE warm-up. The HAM clock governor un-throttles the PE after
            # ~3.4us of sustained busy-ness, and RE-throttles (half clock)
            # on windows with significant PE idle time. Starting the busy
            # window before DMA can feed the PE causes a warm->cold->warm
            # oscillation that costs more than it saves (measured), so
            # warm-ups 4..7 use the first state-chunk tile as (garbage)
            # stationary data, gating them on that DMA -- the busy window
            # begins when real data lands and stays unbroken.
            def warmup():
                pwarm = pp.tile([128, NT], F32, tag="mu", name="pwarm", bufs=2)
                for i in range(3):
                    nc.tensor.matmul(pwarm[:], scr[:, 0:128], scr[:],
                                     start=(i == 0), stop=False)
                for i in range(4):
                    nc.tensor.matmul(pwarm[:], s0t[0][0][0:128, 0:128],
                                     scr[:], start=False, stop=(i == 3))
                # consume so the warm-up group isn't dead-code eliminated
                nc.vector.tensor_scalar_max(scr[0:1, 0:1], pwarm[0:1, 0:1],
                                            0.0)
                # dummy sigmoid: forces the sigmoid table set (which also
                # holds relu/tanh) to load during the initial DMA wait
                # instead of on the first router activation (~1.3us)
                warm = wpool.tile([1, 2], F32, tag="warm")
                nc.vector.memset(warm[:], 0.0)
                nc.scalar.activation(warm[0:1, 0:1], warm[0:1, 1:2],
                                     AF.Sigmoid)

            def blend_g1(t):
                pg1 = pp.tile([BH, NT], F32, tag="g1", name=f"pg1_{t}", bufs=1)
                for ci in range(3):
                    nc.tensor.matmul(pg1[:], W(wd["bl1"][ci]), sv(t, ci),
                                     start=(ci == 0), stop=(ci == 2))
                g1[t] = gpool.tile([BH, NT], DT, tag="g1", name=f"g1_{t}")
                nc.scalar.activation(g1[t][:], pg1[:], AF.Relu)

            def blend_g2(t):
                pg2 = pp.tile([BH, NT], F32, tag="gd", name=f"pg2_{t}", bufs=2)
                nc.tensor.matmul(pg2[:], W(wd["bl2"][0]), g1[t][:],
                                 start=True, stop=True)
                g2[t] = gpool.tile([BH, NT], DT, tag="g2", name=f"g2_{t}")
                nc.scalar.activation(g2[t][:], pg2[:], AF.Relu, bias=W(wd["bb2"]))

            def blend_d(t):
                pd = pp.tile([128, NT], F32, tag="gd", name=f"pd_{t}", bufs=2)
                nc.tensor.matmul(pd[:], W(wd["wd"][0]), g2[t][:],
                                 start=True, stop=True)
                p = t // 2
                if t % 2 == 0:
                    c0[p] = cpool.tile([128, 2 * NT], DT, tag="c0", name=f"c0_{p}")
                nc.scalar.activation(c0[p][:, psl(t)], pd[:], AF.Sigmoid,
                                     bias=W(wd["bd"]))
                # scaled states for tile t, all three chunks in one tile
                # (column band per chunk) so consumers carry one dependency
                # edge instead of three; chunk 0 on DVE, chunks 1/2 on
                # GpSimd (SBUF-only). Row 120 of chunk 2 becomes c0 via the
                # ones-row of xs.
                sct = scpool.tile([128, 3 * NT], DT, tag="sc", name=f"sc_{t}")
                for ci, (k0, k1) in enumerate(KCH):
                    eng = nc.vector if ci == 0 else nc.gpsimd
                    eng.tensor_mul(sct[0 : k1 - k0, ci * NT : (ci + 1) * NT],
                                   sv(t, ci), c0[p][0 : k1 - k0, psl(t)])
                sc[t] = sct

            ph1s = {}
            ph2s = {}
            pmus = {}

            def exp_l1_base(t):
                h1[t] = hpool.tile([128, 2 * NT], DT, tag="h1", name=f"h1_{t}")
                h1c[t] = hpool.tile([128, 2 * NT], DT, tag="h1c", name=f"h1c_{t}")
                for m in range(2):
                    ph = pp.tile([128, NT], F32, tag=f"h1{m}", name=f"ph1{m}_{t}")
                    ph1s[(t, m)] = ph
                    for ci in range(3):
                        nc.tensor.matmul(ph[:], W(wd["e1b"][m][ci]), sv(t, ci),
                                         start=(ci == 0), stop=False)

            def exp_l1_diff(t):
                for m in range(2):
                    ph = ph1s.pop((t, m))
                    for ci, (k0, k1) in enumerate(KCH):
                        nc.tensor.matmul(ph[:], W(wd["e1d"][m][ci]),
                                         sc[t][0 : k1 - k0,
                                               ci * NT : (ci + 1) * NT],
                                         start=False, stop=(ci == 2))
                    hs = slice(m * NT, (m + 1) * NT)
                    nc.vector.tensor_scalar(h1[t][:, hs], ph[:], 0.0,
                                            vbt[:, m : m + 1],
                                            ALU.max, ALU.add)
                    nc.vector.tensor_mul(h1c[t][:, hs], h1[t][:, hs],
                                         c0[t // 2][:, psl(t)])

            def exp_l1(t):
                exp_l1_base(t)
                exp_l1_diff(t)

            def blend_g1_l1base(t):
                # prologue-only: emit the g1 and expert-L1 base matmuls
                # chunk-major, so the PE consumes each state chunk as its
                # DMA lands instead of stalling on the last chunk
                pg1 = pp.tile([BH, NT], F32, tag="g1", name=f"pg1_{t}", bufs=1)
                h1[t] = hpool.tile([128, 2 * NT], DT, tag="h1", name=f"h1_{t}")
                h1c[t] = hpool.tile([128, 2 * NT], DT, tag="h1c", name=f"h1c_{t}")
                phs = [pp.tile([128, NT], F32, tag=f"h1{m}", name=f"ph1{m}_{t}")
                       for m in range(2)]
                for m in range(2):
                    ph1s[(t, m)] = phs[m]
                for ci in range(3):
                    nc.tensor.matmul(pg1[:], W(wd["bl1"][ci]), sv(t, ci),
                                     start=(ci == 0), stop=(ci == 2))
                    for m in range(2):
                        nc.tensor.matmul(phs[m][:], W(wd["e1b"][m][ci]),
                                         sv(t, ci), start=(ci == 0), stop=False)
                g1[t] = gpool.tile([BH, NT], DT, tag="g1", name=f"g1_{t}")
                nc.scalar.activation(g1[t][:], pg1[:], AF.Relu)

            def exp_l2_mm(t):
                ph2s[t] = pp.tile([128, NT], F32, tag="h2", name=f"ph2_{t}")
                ph2 = ph2s[t]
                nc.tensor.matmul(ph2[:], W(wd["e2b"][0]), h1[t][:, 0:NT],
                                 start=True, stop=False)
                nc.tensor.matmul(ph2[:], W(wd["e2b"][1]), h1[t][:, NT : 2 * NT],
                                 start=False, stop=False)
                nc.tensor.matmul(ph2[:], W(wd["e2d"][0]), h1c[t][:, 0:NT],
                                 start=False, stop=False)
                nc.tensor.matmul(ph2[:], W(wd["e2d"][1]), h1c[t][:, NT : 2 * NT],
                                 start=False, stop=True)

            def exp_l3_mm(t):
                # one matmul yields both expert heads: rows 0:17 = Wm1 h2,
                # rows 32:49 = dWm h2
                pmus[t] = pp.tile([32 + NA, NT], F32, tag="mu", name=f"pmu_{t}", bufs=2)
                nc.tensor.matmul(pmus[t][:], W(wd["em"][0]), h2[t][:],
                                 start=True, stop=True)

            def exp_l2_post(t):
                ph2 = ph2s.pop(t)
                h2[t] = hpool.tile([128, NT], DT, tag="h2", name=f"h2_{t}")
                nc.scalar.activation(h2[t][:], ph2[:], AF.Relu, bias=W(wd["b2"]))
                del g1[t], g2[t], sc[t]
                if t % 2 == 1:
                    if t // 2 in s:
                        del s[t // 2]
                    else:
                        s0t.clear()

            def exp_l3_post(t):
                pmu = pmus.pop(t)
                p = t // 2
                # mu = tanh(y1 + c0*(yd + dbm) + bm1): STT fuses the
                # per-partition dbm add with the c0 multiply
                u = opool.tile([NA, NT], DT, tag="u", name=f"u_{t}")
                nc.vector.scalar_tensor_tensor(
                    u[:], pmu[32 : 32 + NA, :], W(wd["dbm"]),
                    c0[p][0:NA, psl(t)], ALU.add, ALU.mult)
                m = opool.tile([NA, NT], DT, tag="m", name=f"m_{t}")
                nc.vector.tensor_add(m[:], u[:], pmu[0:NA, :])
                mt = opool.tile([NA, NT], DT, tag="mu", name=f"mu_{t}",
                                bufs=6)
                nc.scalar.activation(mt[:], m[:], AF.Tanh, bias=W(wd["bm"]))
                nc.sync.dma_start(out[t * NA : (t + 1) * NA, :], mt[:])
                del h1[t], h1c[t], h2[t]
                if t % 2 == 1:
                    del c0[p]

            def exp_l3(t):
                exp_l3_mm(t)
                exp_l3_post(t)

            def exp_l3_pe(t):
                # out layer with the expert blend computed ON THE PE instead
                # of DVE: h2c = (h2 + vm) * c0 (one STT; dWm vm = dbm makes
                # the diff matmul reproduce c0*dbm), then pmu = Wm1 h2 +
                # dWm h2c accumulates both heads into one PSUM and tanh
                # reads it directly. One extra cheap matmul (+213ns PE)
                # removes the STT+ADD (~1.4us) from the DVE critical path --
                # used for the final tiles, where DVE would otherwise gate
                # the kernel tail.
                p = t // 2
                h2c = opool.tile([128, NT], DT, tag="h2c", name=f"h2c_{t}",
                                 bufs=2)
                nc.vector.scalar_tensor_tensor(
                    h2c[:], h2[t][:], vbt[:, 2:3], c0[p][:, psl(t)],
                    ALU.add, ALU.mult)
                pm = pp.tile([NA, NT], F32, tag="mu", name=f"pmu_{t}", bufs=2)
                nc.tensor.matmul(pm[:], W(wd["em1"]), h2[t][:],
                                 start=True, stop=False)
                nc.tensor.matmul(pm[:], W(wd["emd"]), h2c[:],
                                 start=False, stop=True)
                mt = opool.tile([NA, NT], DT, tag="mu", name=f"mu_{t}",
                                bufs=6)
                nc.scalar.activation(mt[:], pm[:], AF.Tanh, bias=W(wd["bm"]))
                nc.gpsimd.dma_start(out[t * NA : (t + 1) * NA, :], mt[:])
                del h1[t], h1c[t], h2[t]
                if t % 2 == 1:
                    del c0[p]

            def exp_l23_tail(t):
                # last tile: run relu -> h2c -> out-matmuls -> tanh -> DMA
                # in two half-width pipelined passes (PE-blend as above) so
                # the final serial chain is short and PE gaps stay small
                ph2 = ph2s.pop(t)
                p = t // 2
                for h in range(2):
                    cs = slice(h * (NT // 2), (h + 1) * (NT // 2))
                    co = slice((t % 2) * NT + h * (NT // 2),
                               (t % 2) * NT + (h + 1) * (NT // 2))
                    h2h = hpool.tile([128, NT // 2], DT, tag="h2s",
                                     name=f"h2s{h}_{t}", bufs=2)
                    nc.scalar.activation(h2h[:], ph2[:, cs], AF.Relu,
                                         bias=W(wd["b2"]))
                    h2ch = opool.tile([128, NT // 2], DT, tag="h2c",
                                      name=f"h2c{h}_{t}", bufs=2)
                    nc.vector.scalar_tensor_tensor(
                        h2ch[:], h2h[:], vbt[:, 2:3], c0[p][:, co],
                        ALU.add, ALU.mult)
                    pm = pp.tile([NA, NT // 2], F32, tag="mu",
                                 name=f"pmu{h}_{t}", bufs=2)
                    nc.tensor.matmul(pm[:], W(wd["em1"]), h2h[:],
                                     start=True, stop=False)
                    nc.tensor.matmul(pm[:], W(wd["emd"]), h2ch[:],
                                     start=False, stop=True)
                    mt = opool.tile([NA, NT // 2], DT, tag="mu",
                                    name=f"mt{h}_{t}", bufs=6)
                    nc.scalar.activation(mt[:], pm[:], AF.Tanh, bias=W(wd["bm"]))
                    nc.gpsimd.dma_start(
                        out[t * NA : (t + 1) * NA,
                            h * (NT // 2) : (h + 1) * (NT // 2)], mt[:])
                del g1[t], g2[t], sc[t], h1[t], h1c[t]
                if t % 2 == 1:
                    if t // 2 in s:
                        del s[t // 2]
                    else:
                        s0t.clear()
                    del c0[p]

            # -------- software-pipelined emission --------
            # prologue covers tiles 0..3 blend chains and tiles 0..1 expert
            # L1 (iterations t=0,1 of the steady loop). The expert-L1 BASE
            # matmuls (no c0 dependency) pack the PE queue early while the
            # DIFF matmuls sit far enough back that the sigmoid->sct chain
            # of their tile has drained by the time the PE reaches them.
            # DMA issue order = DMA priority (16 engines round-robin over all
            # outstanding transfers): first the weights/states the prologue
            # needs soonest; the pair-2/3 transfers are deferred into the
            # prologue so they don't steal bandwidth from tiles 0/1.
            # Early DMA priority. The 16 engines share bandwidth round-robin
            # over every outstanding transfer, so tile 0 lands fastest when
            # little else is in flight: only wk section A rides the gpsimd
            # ring (pushed instantly, lands with t0c0); all other early
            # transfers are metered out through the serial DIRECT2D issues
            # of the Sync queue in NEED order, with four on the Scalar queue
            # (which must be free again before the first relu).
            dma_in_tile(0, qs=(nc.sync, nc.scalar, nc.sync))
            wk_dma(1, nc.scalar)
            dma_in_tile(1, qs=(nc.sync, nc.scalar, nc.sync))
            wk_dma(2, nc.scalar)
            wk_dma(3, nc.sync)
            warmup()
            dma_in(1)
            dma_in(2)
            dma_in(3, (0,))
            nc.sync.dma_start(vbt[:], vb[:])
            blend_g1_l1base(0)
            blend_g2(0)
            blend_g1_l1base(1)
            blend_d(0)
            blend_g2(1)
            blend_g1(2)
            blend_d(1)
            exp_l1_diff(0)
            blend_g2(2)
            blend_g1(3)
            exp_l1_diff(1)
            blend_d(2)
            exp_l2_mm(0)
            blend_g2(3)
            blend_d(3)
            exp_l2_post(0)
            # steady state: iteration t runs L1(t), L2(t-1), L3(t-2) and
            # the blend MLP of t+2 spliced between expert blocks; the last
            # two tiles run one iteration early so the final serial chain
            # overlaps real matmuls instead of dangling at the end
            for t in range(2, T - 2):
                if t % 2 == 1 and (t + 5) // 2 < T // 2:
                    dma_in((t + 5) // 2, (0,))
                if t % 2 == 0 and 2 <= (t + 4) // 2 < T // 2:
                    dma_in((t + 4) // 2, (1, 2))
                if t + 2 < T:
                    blend_g1(t + 2)
                exp_l1(t)
                if t + 2 < T:
                    blend_g2(t + 2)
                exp_l2_mm(t - 1)
                exp_l3_mm(t - 2)
                exp_l2_post(t - 1)
                exp_l3_post(t - 2)
                if t + 2 < T:
                    blend_d(t + 2)
            exp_l1(T - 2)
            exp_l2_mm(T - 3)
            exp_l3_mm(T - 4)
            exp_l2_post(T - 3)
            exp_l3_post(T - 4)
            exp_l1(T - 1)
            exp_l2_mm(T - 2)
            exp_l3_pe(T - 3)
            exp_l2_post(T - 2)
            exp_l2_mm(T - 1)
            exp_l3_pe(T - 2)
            exp_l23_tail(T - 1)
    nc.finalize()
    return nc


_CACHE = {}


def _make_in_maps(inputs):
    states = np.asarray(inputs["states"], np.float32)
    pack = _Pack()
    wdesc = _prep_weights(
        pack,
        *[
            np.asarray(inputs[k], np.float32)
            for k in ("bw1", "bb1", "bw2", "bb2", "bwo", "bbo",
                      "ew1", "eb1", "ew2", "eb2", "ewm", "ebm")
        ],
    )
    wdata = pack.data().astype(DT_NP)  # [128, wcols]
    vbd = np.ascontiguousarray(wdesc["v2h"])
    in_maps = []
    for c in range(N_CORES):
        shard = states[c * BS : (c + 1) * BS]  # [BS, NI]
        xs = np.empty((NIA, BS), np.float32)
        xs[:NI] = shard.T
        xs[NI] = 1.0
        in_maps.append({"xs": xs.astype(DT_NP), "wk": wdata, "vb": vbd})
    return wdesc, wdata, in_maps


def kernel(**inputs) -> np.ndarray:
    wdesc, wdata, in_maps = _make_in_maps(inputs)

    if "nc" not in _CACHE:
        _CACHE["nc"] = _build(wdesc, wdata.shape[1])
    nc = _CACHE["nc"]

    res = run_bass_kernel_spmd(nc, in_maps, core_ids=list(range(N_CORES)))
    out = np.empty((B, NA), np.float32)
    for c in range(N_CORES):
        o = np.asarray(res.results[c]["out"], dtype=np.float32)  # [T*NA, NT]
        out[c * BS : (c + 1) * BS] = (
            o.reshape(T, NA, NT).transpose(0, 2, 1).reshape(BS, NA)
        )
    return out



# revision 27
# speedup vs baseline: 1.0174x; 1.0174x over previous
"""Trainium2 Bass kernel for the blended-MoE actor network.

Math: reference computes, per sample,
    g1 = relu(bw1 @ s + bb1); g2 = relu(bw2 @ g1 + bb2)
    c  = softmax(bwo @ g2 + bbo)            # 2 experts
    h1 = relu(blend(W1_e, s)); h2 = relu(blend(W2_e, h1))
    mu = tanh(blend(Wm_e, h2))
with blend(W_e, x) = sum_e c_e (W_e x + b_e).

Since NE=2 and c0+c1=1:  c0 = sigmoid((bwo[0]-bwo[1]) @ g2 + dbo)  and
    blend(W_e, x) = W_1 x + b_1 + c0 * (dW x + db),  dW = W_0-W_1.
For L1/L2 the c0 * (dW x) term is computed by scaling the matmul INPUT
per-sample (x_c = C0 .* x) so both expert contributions accumulate into
one PSUM group. Rank-1 bias terms ride along: an appended ones-row on
the states makes row 376 of the scaled states equal c0 (feeding the db
column of the augmented diff weights); for L2 the bias diff is folded
into h1 via v solving dW2 v = db2 (h1c = C0 .* (h1 + v), base-path
constant compensated in the h2 bias).
The OUT layer blends at the output instead: one [128,34] stationary
tile [Wm1.T | dWm.T] yields both expert heads from a single matmul;
mu = tanh(y1 + c0*(yd + dbm) + bm1) with the inner term one
scalar_tensor_tensor op (per-partition dbm, tensor c0).

Layout: activations are [features, batch] on-chip (host pre-transposes
states and appends a ones row); batch tiles of N=512 (one PSUM bank per
matmul). The router logit-diff matmul uses wd replicated across 128
output columns so its PSUM output holds the logit diff in every
partition row: one sigmoid yields the broadcast C0 tile for free.

The per-tile dataflow is a serial chain (blend MLP -> C0 -> scaled
inputs -> expert layers), so instructions are emitted in an explicit
software-pipelined order across batch tiles -- the TensorEngine stream
interleaves expert layers of tiles t, t-1, t-2 with the blend MLP of
tile t+2, keeping the PE free of cross-engine round-trip stalls.

Engine balance per tile (measured ns): PE 22 matmuls ~4700, DVE
(h1 relu+v x2, h1c x2, sc chunk0, L3 blend x2) ~4000, GpSimd (sc
chunks 1/2) ~2600, ACT (g1/g2/h2 relu, sigmoid, tanh) ~3450.

Output is written bf16 in a tile-contiguous DRAM layout ([T*17, 512])
so the final DMA is a small contiguous burst instead of a slow
17-row strided scatter; the host reassembles.

Sharding: pure data parallel over 8 cores (batch 65536 -> 8 x 8192).
"""

import ml_dtypes
import numpy as np

import concourse.bass as bass
import concourse.mybir as mybir
import concourse.tile as tile
from concourse import bacc
from concourse.bass_utils import run_bass_kernel_spmd

N_CORES = 8
B = 65536
BS = B // N_CORES  # 8192 per core
NI = 376  # state features
NIA = NI + 1  # + ones row
NA = 17  # actions
BH = 128  # blending hidden
NT = 512  # batch tile (matmul free dim, one PSUM bank)
T = BS // NT  # 16 tiles per core

F32 = mybir.dt.float32
# bf16 compute: matmul streams 1 col/cycle warm (f32r measures 2), DVE
# tensor_tensor gets 2x mode, DMA bytes halve. rel err ~8e-3 << 2e-2.
DT = mybir.dt.bfloat16
DT_NP = ml_dtypes.bfloat16
F8 = mybir.dt.float8e4
F8_NP = ml_dtypes.float8_e4m3fn

AF = mybir.ActivationFunctionType
ALU = mybir.AluOpType
KCH = ((0, 128), (128, 256), (256, NIA))  # K chunks of the state dim


# ---------------------------------------------------------------- weights
# All stationary operands are packed into one [128, WCOLS] host array;
# each lhsT is a column slice [0:K, off:off+M]. Rows >= K are zero.


class _Pack:
    def __init__(self):
        self.cols = []
        self.off = 0

    def add(self, arr):  # arr [K, M] -> returns (off, K, M)
        k, m = arr.shape
        assert k <= 128
        a = np.zeros((128, m), np.float32)
        a[:k] = arr
        off = self.off
        self.cols.append(a)
        self.off += m
        return (off, k, m)

    def data(self):
        return np.concatenate(self.cols, axis=1)


def _prep_weights(p, bw1, bb1, bw2, bb2, bwo, bbo, ew1, eb1, ew2, eb2, ewm, ebm):
    d = {}
    # Pack is CHUNK-MAJOR: section k holds every stationary consuming state
    # chunk k (blend L1 + expert L1 base/diff), so the first matmuls are
    # gated on one small wk section + one state chunk instead of the whole
    # pack. Section 3 holds everything downstream of the state chunks.
    w1a = np.concatenate([bw1.T, bb1[None, :]], axis=0)  # [377, 128]
    e1b = np.concatenate([ew1[1].T, eb1[1][None, :]], axis=0)  # [377, 256]
    e1d = np.concatenate([(ew1[0] - ew1[1]).T, (eb1[0] - eb1[1])[None, :]], axis=0)
    d["bl1"] = [None] * 3
    d["e1b"] = [[None] * 3 for _ in range(2)]
    d["e1d"] = [[None] * 3 for _ in range(2)]
    d["splits"] = []
    for ci, (k0, k1) in enumerate(KCH):
        d["bl1"][ci] = p.add(w1a[k0:k1])
        for m in range(2):
            d["e1b"][m][ci] = p.add(e1b[k0:k1, m * 128 : m * 128 + 128])
            d["e1d"][m][ci] = p.add(e1d[k0:k1, m * 128 : m * 128 + 128])
        d["splits"].append(p.off)
    # --- section 3: blend L2/router + expert L2 / out weights
    d["bl2"] = [p.add(bw2.T)]
    d["bb2"] = p.add(bb2[:, None])
    # router logit diff, replicated to 128 output columns
    wd = (bwo[0] - bwo[1])[:, None]
    d["wd"] = [p.add(np.repeat(wd, 128, axis=1))]
    d["bd"] = p.add(np.full((128, 1), bbo[0] - bbo[1], np.float32))
    e2b = ew2[1].T  # [256, 128]
    e2d = (ew2[0] - ew2[1]).T
    d["e2b"] = [p.add(e2b[0:128]), p.add(e2b[128:256])]
    d["e2d"] = [p.add(e2d[0:128]), p.add(e2d[128:256])]
    # rank-1 bias c0*db2 folded into h1: with v solving (ew2[0]-ew2[1]) v =
    # db2 (min-norm), h1c = C0 .* (h1 + v) makes the e2d matmul reproduce
    # c0*db2 exactly; the constant (ew2[1] @ v) on the base path moves into
    # the h2-relu bias. Kills one K=1 matmul per tile.
    E = (ew2[0] - ew2[1]).astype(np.float64)  # [128, 256]
    v = np.linalg.lstsq(E, (eb2[0] - eb2[1]).astype(np.float64), rcond=None)[0]
    v = v.astype(np.float32)
    d["b2"] = p.add((eb2[1] - ew2[1] @ v)[:, None])
    # same rank-1 trick for the out layer (used by the tail tiles where the
    # output blend runs on the PE): vm solves dWm vm = dbm, h2c=(h2+vm)*c0
    Em = (ewm[0] - ewm[1]).astype(np.float64)  # [17, 128]
    vm = np.linalg.lstsq(Em, (ebm[0] - ebm[1]).astype(np.float64), rcond=None)[0]
    d["v2h"] = np.stack([v[0:128], v[128:256], vm.astype(np.float32)], axis=1)
    # expert out: both heads from one stationary tile [128, 49]; the diff
    # head sits at output partition 32 (PSUM reads must start 32-aligned)
    d["em"] = [p.add(np.concatenate(
        [ewm[1].T, np.zeros((128, 32 - NA), np.float32),
         (ewm[0] - ewm[1]).T], axis=1))]
    off_em = d["em"][0][0]
    d["em1"] = (off_em, 128, NA)
    d["emd"] = (off_em + 32, 128, NA)
    d["dbm"] = p.add((ebm[0] - ebm[1])[:, None])  # [17, 1]
    d["bm"] = p.add(ebm[1][:, None])  # [17, 1]
    return d


# ---------------------------------------------------------------- kernel


def _build(wd, wcols):
    nc = bacc.Bacc("TRN2", target_bir_lowering=False, debug=False,
                   num_devices=N_CORES)
    xs = nc.declare_dram_parameter("xs", [NIA, BS], DT, isOutput=False)
    wk = nc.declare_dram_parameter("wk", [128, wcols], DT, isOutput=False)
    vb = nc.declare_dram_parameter("vb", [128, 3], F32, isOutput=False)
    out = nc.declare_dram_parameter("out", [T * NA, NT], DT, isOutput=True)

    splits = wd["splits"]  # chunk-section boundaries in the pack

    with tile.TileContext(nc) as tc:
        with (
            tc.tile_pool(name="wpool", bufs=1) as wpool,
            tc.tile_pool(name="spool", bufs=3) as spool,
            tc.tile_pool(name="scpool", bufs=4) as scpool,
            tc.tile_pool(name="gpool", bufs=3) as gpool,
            tc.tile_pool(name="cpool", bufs=5) as cpool,
            tc.tile_pool(name="hpool", bufs=3) as hpool,
            tc.tile_pool(name="opool", bufs=3) as opool,
            tc.tile_pool(name="psum", bufs=1, space="PSUM") as pp,
        ):
            # Weight pack arrives as four tiles, chunk-major: section k holds
            # every stationary consuming state chunk k, so the first matmul
            # block is gated on ~160KB instead of the whole pack. All weight
            # DMAs ride the (otherwise idle) GpSimd queue so the Sync queue
            # serves the state-chunk DMAs back to back.
            scr = wpool.tile([128, NT], DT, tag="scr")
            nc.vector.memset(scr[:], 0.0)
            ends = splits + [wcols]
            begs = [0] + splits
            wkts = []
            for i, (b, e) in enumerate(zip(begs, ends)):
                wkts.append(wpool.tile([128, e - b], DT, name=f"wkt{i}",
                                       tag=f"wkt{i}"))
            nc.sync.dma_start(wkts[0][:], wk[:, begs[0] : ends[0]])
            vbt = wpool.tile([128, 3], F32)

            def wk_dma(i, q=None):
                (q or nc.gpsimd).dma_start(wkts[i][:], wk[:, begs[i] : ends[i]])

            def W(desc):
                off, k, m = desc
                for i, e in enumerate(ends):
                    if off + m <= e:
                        return wkts[i][0:k, off - begs[i] : off - begs[i] + m]
                raise AssertionError("bad pack slice")

            # per-tile (t) and per-pair (p = t//2) live tensors; pairs are
            # 1024 wide so DMA and GpSimd muls run at half the instruction
            # and semaphore count. Pair 0 is loaded per-TILE instead: the
            # 16 DMA engines round-robin over every outstanding transfer,
            # so the first tile's data lands ~2x sooner when its transfers
            # are half-size and first in the queue.
            s = {}
            s0t = {}  # s0t[t][ci] for tiles 0/1
            sc = {}
            c0 = {}
            g1 = {}
            g2 = {}
            h1 = {}
            h1c = {}
            h2 = {}

            def psl(t):  # slice of tile t within its pair tile
                return slice((t % 2) * NT, (t % 2 + 1) * NT)

            def sv(t, ci):  # state chunk view for tile t
                if t < 2:
                    return s0t[t][ci][:]
                return s[t // 2][ci][:, psl(t)]

            def dma_q(ci):
                return nc.sync

            def dma_in_tile(t, chunks=(0, 1, 2), qs=None):
                s0t.setdefault(t, [None, None, None])
                for ci in chunks:
                    k0, k1 = KCH[ci]
                    st = spool.tile([k1 - k0, NT], DT, tag=f"st{t}_{ci}",
                                    name=f"st{t}_{ci}", bufs=1)
                    q = qs[ci] if qs else dma_q(ci)
                    q.dma_start(st[:], xs[k0:k1, t * NT : (t + 1) * NT])
                    s0t[t][ci] = st

            def dma_in(p, chunks=(0, 1, 2)):
                # pair p: columns [p*2NT, (p+1)*2NT); chunks may be staggered
                # across iterations to spread SBUF write-port contention
                if p not in s:
                    s[p] = [None, None, None]
                for ci in chunks:
                    k0, k1 = KCH[ci]
                    st = spool.tile([k1 - k0, 2 * NT], DT, tag=f"s{ci}",
                                    name=f"s{ci}_{p}", bufs=5)
                    dma_q(ci).dma_start(
                        st[:], xs[k0:k1, p * 2 * NT : (p + 1) * 2 * NT])
                    s[p][ci] = st

            # PE warm-up. The HAM clock governor un-throttles the PE after
            # ~3.4us of sustained busy-ness, and RE-throttles (half clock)
            # on windows with significant PE idle time. Starting the busy
            # window before DMA can feed the PE causes a warm->cold->warm
            # oscillation that costs more than it saves (measured), so
            # warm-ups 4..7 use the first state-chunk tile as (garbage)
            # stationary data, gating them on that DMA -- the busy window
            # begins when real data lands and stays unbroken.
            def warmup():
                pwarm = pp.tile([128, NT], F32, tag="mu", name="pwarm", bufs=2)
                for i in range(3):
                    nc.tensor.matmul(pwarm[:], scr[:, 0:128], scr[:],
                                     start=(i == 0), stop=False)
                for i in range(4):
                    nc.tensor.matmul(pwarm[:], s0t[0][0][0:128, 0:128],
                                     scr[:], start=False, stop=(i == 3))
                # consume so the warm-up group isn't dead-code eliminated
                nc.vector.tensor_scalar_max(scr[0:1, 0:1], pwarm[0:1, 0:1],
                                            0.0)
                # dummy sigmoid: forces the sigmoid table set (which also
                # holds relu/tanh) to load during the initial DMA wait
                # instead of on the first router activation (~1.3us)
                warm = wpool.tile([1, 2], F32, tag="warm")
                nc.vector.memset(warm[:], 0.0)
                nc.scalar.activation(warm[0:1, 0:1], warm[0:1, 1:2],
                                     AF.Sigmoid)

            def blend_g1(t):
                pg1 = pp.tile([BH, NT], F32, tag="g1", name=f"pg1_{t}", bufs=1)
                for ci in range(3):
                    nc.tensor.matmul(pg1[:], W(wd["bl1"][ci]), sv(t, ci),
                                     start=(ci == 0), stop=(ci == 2))
                g1[t] = gpool.tile([BH, NT], DT, tag="g1", name=f"g1_{t}")
                nc.scalar.activation(g1[t][:], pg1[:], AF.Relu)

            def blend_g2(t):
                pg2 = pp.tile([BH, NT], F32, tag="gd", name=f"pg2_{t}", bufs=2)
                nc.tensor.matmul(pg2[:], W(wd["bl2"][0]), g1[t][:],
                                 start=True, stop=True)
                g2[t] = gpool.tile([BH, NT], DT, tag="g2", name=f"g2_{t}")
                nc.scalar.activation(g2[t][:], pg2[:], AF.Relu, bias=W(wd["bb2"]))

            def blend_d(t):
                pd = pp.tile([128, NT], F32, tag="gd", name=f"pd_{t}", bufs=2)
                nc.tensor.matmul(pd[:], W(wd["wd"][0]), g2[t][:],
                                 start=True, stop=True)
                p = t // 2
                if t % 2 == 0:
                    c0[p] = cpool.tile([128, 2 * NT], DT, tag="c0", name=f"c0_{p}")
                nc.scalar.activation(c0[p][:, psl(t)], pd[:], AF.Sigmoid,
                                     bias=W(wd["bd"]))
                # scaled states for tile t, all three chunks in one tile
                # (column band per chunk) so consumers carry one dependency
                # edge instead of three; chunk 0 on DVE, chunks 1/2 on
                # GpSimd (SBUF-only). Row 120 of chunk 2 becomes c0 via the
                # ones-row of xs.
                sct = scpool.tile([128, 3 * NT], DT, tag="sc", name=f"sc_{t}")
                for ci, (k0, k1) in enumerate(KCH):
                    eng = nc.vector if ci == 0 else nc.gpsimd
                    eng.tensor_mul(sct[0 : k1 - k0, ci * NT : (ci + 1) * NT],
                                   sv(t, ci), c0[p][0 : k1 - k0, psl(t)])
                sc[t] = sct

            ph1s = {}
            ph2s = {}
            pmus = {}

            def exp_l1_base(t):
                h1[t] = hpool.tile([128, 2 * NT], DT, tag="h1", name=f"h1_{t}")
                h1c[t] = hpool.tile([128, 2 * NT], DT, tag="h1c", name=f"h1c_{t}")
                for m in range(2):
                    ph = pp.tile([128, NT], F32, tag=f"h1{m}", name=f"ph1{m}_{t}")
                    ph1s[(t, m)] = ph
                    for ci in range(3):
                        nc.tensor.matmul(ph[:], W(wd["e1b"][m][ci]), sv(t, ci),
                                         start=(ci == 0), stop=False)

            def exp_l1_diff(t):
                for m in range(2):
                    ph = ph1s.pop((t, m))
                    for ci, (k0, k1) in enumerate(KCH):
                        nc.tensor.matmul(ph[:], W(wd["e1d"][m][ci]),
                                         sc[t][0 : k1 - k0,
                                               ci * NT : (ci + 1) * NT],
                                         start=False, stop=(ci == 2))
                    hs = slice(m * NT, (m + 1) * NT)
                    nc.vector.tensor_scalar(h1[t][:, hs], ph[:], 0.0,
                                            vbt[:, m : m + 1],
                                            ALU.max, ALU.add)
                    nc.vector.tensor_mul(h1c[t][:, hs], h1[t][:, hs],
                                         c0[t // 2][:, psl(t)])

            def exp_l1(t):
                exp_l1_base(t)
                exp_l1_diff(t)

            def blend_g1_l1base(t):
                # prologue-only: emit the g1 and expert-L1 base matmuls
                # chunk-major, so the PE consumes each state chunk as its
                # DMA lands instead of stalling on the last chunk
                pg1 = pp.tile([BH, NT], F32, tag="g1", name=f"pg1_{t}", bufs=1)
                h1[t] = hpool.tile([128, 2 * NT], DT, tag="h1", name=f"h1_{t}")
                h1c[t] = hpool.tile([128, 2 * NT], DT, tag="h1c", name=f"h1c_{t}")
                phs = [pp.tile([128, NT], F32, tag=f"h1{m}", name=f"ph1{m}_{t}")
                       for m in range(2)]
                for m in range(2):
                    ph1s[(t, m)] = phs[m]
                for ci in range(3):
                    nc.tensor.matmul(pg1[:], W(wd["bl1"][ci]), sv(t, ci),
                                     start=(ci == 0), stop=(ci == 2))
                    for m in range(2):
                        nc.tensor.matmul(phs[m][:], W(wd["e1b"][m][ci]),
                                         sv(t, ci), start=(ci == 0), stop=False)
                g1[t] = gpool.tile([BH, NT], DT, tag="g1", name=f"g1_{t}")
                nc.scalar.activation(g1[t][:], pg1[:], AF.Relu)

            def exp_l2_mm(t):
                ph2s[t] = pp.tile([128, NT], F32, tag="h2", name=f"ph2_{t}")
                ph2 = ph2s[t]
                nc.tensor.matmul(ph2[:], W(wd["e2b"][0]), h1[t][:, 0:NT],
                                 start=True, stop=False)
                nc.tensor.matmul(ph2[:], W(wd["e2b"][1]), h1[t][:, NT : 2 * NT],
                                 start=False, stop=False)
                nc.tensor.matmul(ph2[:], W(wd["e2d"][0]), h1c[t][:, 0:NT],
                                 start=False, stop=False)
                nc.tensor.matmul(ph2[:], W(wd["e2d"][1]), h1c[t][:, NT : 2 * NT],
                                 start=False, stop=True)

            def exp_l3_mm(t):
                # one matmul yields both expert heads: rows 0:17 = Wm1 h2,
                # rows 32:49 = dWm h2
                pmus[t] = pp.tile([32 + NA, NT], F32, tag="mu", name=f"pmu_{t}", bufs=2)
                nc.tensor.matmul(pmus[t][:], W(wd["em"][0]), h2[t][:],
                                 start=True, stop=True)

            def exp_l2_post(t):
                ph2 = ph2s.pop(t)
                h2[t] = hpool.tile([128, NT], DT, tag="h2", name=f"h2_{t}")
                nc.scalar.activation(h2[t][:], ph2[:], AF.Relu, bias=W(wd["b2"]))
                del g1[t], g2[t], sc[t]
                if t % 2 == 1:
                    if t // 2 in s:
                        del s[t // 2]
                    else:
                        s0t.clear()

            def exp_l3_post(t):
                pmu = pmus.pop(t)
                p = t // 2
                # mu = tanh(y1 + c0*(yd + dbm) + bm1): STT fuses the
                # per-partition dbm add with the c0 multiply
                u = opool.tile([NA, NT], DT, tag="u", name=f"u_{t}")
                nc.vector.scalar_tensor_tensor(
                    u[:], pmu[32 : 32 + NA, :], W(wd["dbm"]),
                    c0[p][0:NA, psl(t)], ALU.add, ALU.mult)
                m = opool.tile([NA, NT], DT, tag="m", name=f"m_{t}")
                nc.vector.tensor_add(m[:], u[:], pmu[0:NA, :])
                mt = opool.tile([NA, NT], DT, tag="mu", name=f"mu_{t}",
                                bufs=6)
                nc.scalar.activation(mt[:], m[:], AF.Tanh, bias=W(wd["bm"]))
                nc.sync.dma_start(out[t * NA : (t + 1) * NA, :], mt[:])
                del h1[t], h1c[t], h2[t]
                if t % 2 == 1:
                    del c0[p]

            def exp_l3(t):
                exp_l3_mm(t)
                exp_l3_post(t)

            def exp_l3_pe(t):
                # out layer with the expert blend computed ON THE PE instead
                # of DVE: h2c = (h2 + vm) * c0 (one STT; dWm vm = dbm makes
                # the diff matmul reproduce c0*dbm), then pmu = Wm1 h2 +
                # dWm h2c accumulates both heads into one PSUM and tanh
                # reads it directly. One extra cheap matmul (+213ns PE)
                # removes the STT+ADD (~1.4us) from the DVE critical path --
                # used for the final tiles, where DVE would otherwise gate
                # the kernel tail.
                p = t // 2
                h2c = opool.tile([128, NT], DT, tag="h2c", name=f"h2c_{t}",
                                 bufs=2)
                nc.vector.scalar_tensor_tensor(
                    h2c[:], h2[t][:], vbt[:, 2:3], c0[p][:, psl(t)],
                    ALU.add, ALU.mult)
                pm = pp.tile([NA, NT], F32, tag="mu", name=f"pmu_{t}", bufs=2)
                nc.tensor.matmul(pm[:], W(wd["em1"]), h2[t][:],
                                 start=True, stop=False)
                nc.tensor.matmul(pm[:], W(wd["emd"]), h2c[:],
                                 start=False, stop=True)
                mt = opool.tile([NA, NT], DT, tag="mu", name=f"mu_{t}",
                                bufs=6)
                nc.scalar.activation(mt[:], pm[:], AF.Tanh, bias=W(wd["bm"]))
                nc.gpsimd.dma_start(out[t * NA : (t + 1) * NA, :], mt[:])
                del h1[t], h1c[t], h2[t]
                if t % 2 == 1:
                    del c0[p]

            def exp_l23_tail(t):
                # last tile: run relu -> h2c -> out-matmuls -> tanh -> DMA
                # in two half-width pipelined passes (PE-blend as above) so
                # the final serial chain is short and PE gaps stay small
                ph2 = ph2s.pop(t)
                p = t // 2
                for h in range(2):
                    cs = slice(h * (NT // 2), (h + 1) * (NT // 2))
                    co = slice((t % 2) * NT + h * (NT // 2),
                               (t % 2) * NT + (h + 1) * (NT // 2))
                    h2h = hpool.tile([128, NT // 2], DT, tag="h2s",
                                     name=f"h2s{h}_{t}", bufs=2)
                    nc.scalar.activation(h2h[:], ph2[:, cs], AF.Relu,
                                         bias=W(wd["b2"]))
                    h2ch = opool.tile([128, NT // 2], DT, tag="h2c",
                                      name=f"h2c{h}_{t}", bufs=2)
                    nc.vector.scalar_tensor_tensor(
                        h2ch[:], h2h[:], vbt[:, 2:3], c0[p][:, co],
                        ALU.add, ALU.mult)
                    pm = pp.tile([NA, NT // 2], F32, tag="mu",
                                 name=f"pmu{h}_{t}", bufs=2)
                    nc.tensor.matmul(pm[:], W(wd["em1"]), h2h[:],
                                     start=True, stop=False)
                    nc.tensor.matmul(pm[:], W(wd["emd"]), h2ch[:],
                                     start=False, stop=True)
                    mt = opool.tile([NA, NT // 2], DT, tag="mu",
                                    name=f"mt{h}_{t}", bufs=6)
                    nc.scalar.activation(mt[:], pm[:], AF.Tanh, bias=W(wd["bm"]))
                    nc.gpsimd.dma_start(
                        out[t * NA : (t + 1) * NA,
                            h * (NT // 2) : (h + 1) * (NT // 2)], mt[:])
                del g1[t], g2[t], sc[t], h1[t], h1c[t]
                if t % 2 == 1:
                    if t // 2 in s:
                        del s[t // 2]
                    else:
                        s0t.clear()
                    del c0[p]

            # -------- software-pipelined emission --------
            # prologue covers tiles 0..3 blend chains and tiles 0..1 expert
            # L1 (iterations t=0,1 of the steady loop). The expert-L1 BASE
            # matmuls (no c0 dependency) pack the PE queue early while the
            # DIFF matmuls sit far enough back that the sigmoid->sct chain
            # of their tile has drained by the time the PE reaches them.
            # DMA issue order = DMA priority (16 engines round-robin over all
            # outstanding transfers): first the weights/states the prologue
            # needs soonest; the pair-2/3 transfers are deferred into the
            # prologue so they don't steal bandwidth from tiles 0/1.
            # Early DMA priority. The 16 engines share bandwidth round-robin
            # over every outstanding transfer, so tile 0 lands fastest when
            # little else is in flight: only wk section A rides the gpsimd
            # ring (pushed instantly, lands with t0c0); all other early
            # transfers are metered out through the serial DIRECT2D issues
            # of the Sync queue in NEED order, with four on the Scalar queue
            # (which must be free again before the first relu).
            dma_in_tile(0, qs=(nc.sync, nc.scalar, nc.sync))
            wk_dma(1, nc.scalar)
            dma_in_tile(1, qs=(nc.sync, nc.scalar, nc.sync))
            wk_dma(2, nc.scalar)
            wk_dma(3, nc.sync)
            warmup()
            dma_in(1)
            dma_in(2)
            dma_in(3, (0,))
            nc.sync.dma_start(vbt[:], vb[:])
            blend_g1_l1base(0)
            blend_g2(0)
            blend_g1_l1base(1)
            blend_d(0)
            blend_g2(1)
            blend_g1(2)
            blend_d(1)
            exp_l1_diff(0)
            blend_g2(2)
            blend_g1(3)
            exp_l1_diff(1)
            blend_d(2)
            exp_l2_mm(0)
            blend_g2(3)
            blend_d(3)
            exp_l2_post(0)
            # steady state: iteration t runs L1(t), L2(t-1), L3(t-2) and
            # the blend MLP of t+2 spliced between expert blocks; the last
            # two tiles run one iteration early so the final serial chain
            # overlaps real matmuls instead of dangling at the end
            for t in range(2, T - 2):
                if t % 2 == 1 and (t + 5) // 2 < T // 2:
                    dma_in((t + 5) // 2, (0,))
                if t % 2 == 0 and 2 <= (t + 4) // 2 < T // 2:
                    dma_in((t + 4) // 2, (1, 2))
                if t + 2 < T:
                    blend_g1(t + 2)
                exp_l1(t)
                if t + 2 < T:
                    blend_g2(t + 2)
                exp_l2_mm(t - 1)
                exp_l3_mm(t - 2)
                exp_l2_post(t - 1)
                exp_l3_post(t - 2)
                if t + 2 < T:
                    blend_d(t + 2)
            exp_l1(T - 2)
            exp_l2_mm(T - 3)
            exp_l3_mm(T - 4)
            exp_l2_post(T - 3)
            exp_l3_post(T - 4)
            exp_l1(T - 1)
            exp_l2_mm(T - 2)
            exp_l3_pe(T - 3)
            exp_l2_post(T - 2)
            exp_l2_mm(T - 1)
            exp_l3_pe(T - 2)
            exp_l23_tail(T - 1)
    nc.finalize()
    return nc


_CACHE = {}


def _make_in_maps(inputs):
    states = np.asarray(inputs["states"], np.float32)
    pack = _Pack()
    wdesc = _prep_weights(
        pack,
        *[
            np.asarray(inputs[k], np.float32)
            for k in ("bw1", "bb1", "bw2", "bb2", "bwo", "bbo",
                      "ew1", "eb1", "ew2", "eb2", "ewm", "ebm")
        ],
    )
    wdata = pack.data().astype(DT_NP)  # [128, wcols]
    vbd = np.ascontiguousarray(wdesc["v2h"])
    in_maps = []
    for c in range(N_CORES):
        shard = states[c * BS : (c + 1) * BS]  # [BS, NI]
        xs = np.empty((NIA, BS), np.float32)
        xs[:NI] = shard.T
        xs[NI] = 1.0
        in_maps.append({"xs": xs.astype(DT_NP), "wk": wdata, "vb": vbd})
    return wdesc, wdata, in_maps


def kernel(**inputs) -> np.ndarray:
    wdesc, wdata, in_maps = _make_in_maps(inputs)

    if "nc" not in _CACHE:
        _CACHE["nc"] = _build(wdesc, wdata.shape[1])
    nc = _CACHE["nc"]

    res = run_bass_kernel_spmd(nc, in_maps, core_ids=list(range(N_CORES)))
    out = np.empty((B, NA), np.float32)
    for c in range(N_CORES):
        o = np.asarray(res.results[c]["out"], dtype=np.float32)  # [T*NA, NT]
        out[c * BS : (c + 1) * BS] = (
            o.reshape(T, NA, NT).transpose(0, 2, 1).reshape(BS, NA)
        )
    return out

